# revision 32
# baseline (speedup 1.0000x reference)
"""Trainium2 Bass kernel for quantized InvertedResidual block (DoReFa fake-quant).

Strategy (v3, all-fp16 matmuls + software pipelining):
- Data-parallel: 32 images -> 4 per core across 8 NeuronCores.
- All matmuls fp16 (1 cycle/row on PE vs 4 for fp32):
  stage1: x split into fp16 hi+lo (22 mantissa bits), packed as K=128
          with duplicated integer weights [w1i; w1i] (w1i = w1q*255,
          exact in fp16) -> one matmul per (group, pixel tile).
  stage2: depthwise 3x3 = 9 diagonal matmuls, integer weights.
  stage3: 1x1 conv with integer weights w3q*255; activations are exact
          integers so products/accumulation are exact in PSUM.
- fp16 magic rounding: ACT computes scale*psum + (bias + 1024) in fp32;
  its fp16 output downcast rounds to the integer grid via RNE
  (v + 1024 in [1024, 2048) has ulp 1 in fp16). Activations are stored
  offset by +1024; the next stage's bias absorbs 1024*sum(weights)
  exactly. One DVE clamp (max 1024, min 1279) per tile finishes the
  quant. Residual uses x' = x - 1024/255 prepped on host.
- ACT/DVE ops cover two 448-pixel tiles at once (PSUM tiles span two
  banks, matmuls write at bank-aligned offsets 0/512) to amortize the
  ~370ns ACT init overhead.
- Software pipeline: image i+1's stage-1 units are interleaved into
  image i's stage-2 stream so the PE never stalls waiting for the
  (slower) ACT drain of stage-1 PSUM tiles; h1/h2 are double-buffered
  by image parity.
"""
import numpy as np

EPS = 1e-5
OFF = np.float32(1024.0)   # fp16 magic offset: [1024, 2048) has ulp 1

B, C, H, W = 32, 64, 56, 56
HID = 384
NCORES = 8
BPC = B // NCORES          # images per core
PIX = H * W                # 3136
PW = W + 2                 # 58
PH = H + 2
PPIX = PW * PH             # 3364
NT = 7                     # pixel tiles per image
TW = PIX // NT             # 448 = 8 rows x 56
ROWS_PT = H // NT          # 8
NG = HID // 128            # 3 channel groups
NU = (NT + 1) // 2         # 4 double-width units per (group, image)

_cache = {}


def _quant_w(w):
    # DoReFa weight fake-quant, computed with jax on CPU so tanh/round are
    # bitwise identical to the reference implementation.
    import jax
    import jax.numpy as jnp
    with jax.default_device(jax.devices('cpu')[0]):
        t = jnp.tanh(jnp.asarray(w, jnp.float32))
        m = jnp.max(jnp.abs(t), axis=(1, 2, 3), keepdims=True)
        wn = t / (2.0 * m) + 0.5
        q = 2.0 * jnp.round(wn * 255.0) / 255.0 - 1.0
        return np.asarray(q, np.float32)


def _build_program():
    import concourse.bass as bass
    import concourse.tile as tile
    from concourse import bacc, mybir

    fp32 = mybir.dt.float32
    f16 = mybir.dt.float16
    nc = bacc.Bacc("TRN2", target_bir_lowering=False, debug=False,
                   enable_asserts=False, num_devices=NCORES)

    xhl = nc.dram_tensor("xhl", [BPC, 128, PIX], f16, kind="ExternalInput").ap()
    xp = nc.dram_tensor("xp", [BPC, 64, PIX], fp32, kind="ExternalInput").ap()
    w1s = nc.dram_tensor("w1s", [128, HID], f16, kind="ExternalInput").ap()
    wdw = nc.dram_tensor("wdw", [128, NG * 9 * 128], f16, kind="ExternalInput").ap()
    w3i = nc.dram_tensor("w3i", [128, NG * 64], f16, kind="ExternalInput").ap()
    s1m = nc.dram_tensor("s1m", [128, NG], fp32, kind="ExternalInput").ap()
    b1m = nc.dram_tensor("b1m", [128, NG], fp32, kind="ExternalInput").ap()
    s2m = nc.dram_tensor("s2m", [128, NG], fp32, kind="ExternalInput").ap()
    b2m = nc.dram_tensor("b2m", [128, NG], fp32, kind="ExternalInput").ap()
    s3m = nc.dram_tensor("s3m", [64, 1], fp32, kind="ExternalInput").ap()
    b3m = nc.dram_tensor("b3m", [64, 1], fp32, kind="ExternalInput").ap()
    wdv = nc.dram_tensor("wdv", [128, NG * 9], fp32, kind="ExternalInput").ap()
    ys = nc.dram_tensor("ys", [BPC, 64, PIX], fp32, kind="ExternalOutput").ap()

    # stage-2 units computed on the vector engines instead of the PE
    # (depthwise tap = per-partition FMA via scalar_tensor_tensor)
    GPS_S2 = set()
    DVE_S2 = {(2, 0), (0, 1), (2, 3)}
    # PE stage-2 units whose tap-0 is seeded into PSUM by the ACT engine:
    # DISABLED — ACT->PSUM seeding gives wrong results on HW and the PSUM
    # write contention slows every matmul down.
    ACT_TAP0 = set()

    mx = mybir.AluOpType.max
    mn = mybir.AluOpType.min
    add = mybir.AluOpType.add
    mult = mybir.AluOpType.mult
    IDENT = mybir.ActivationFunctionType.Identity

    CLO = float(OFF)           # 1024.0
    CHI = float(OFF) + 255.0   # 1279.0
    TAPS = [(dy, dx) for dy in (-1, 0, 1) for dx in (-1, 0, 1)]

    with tile.TileContext(nc) as tc:
        from contextlib import ExitStack
        with ExitStack() as ctx:
            consts = ctx.enter_context(tc.tile_pool(name="consts", bufs=1))
            h1p_pool = ctx.enter_context(tc.tile_pool(name="h1p", bufs=1))
            h2_pool = ctx.enter_context(tc.tile_pool(name="h2", bufs=1))
            x_pool = ctx.enter_context(tc.tile_pool(name="x", bufs=2))
            o_pool = ctx.enter_context(tc.tile_pool(name="o", bufs=2))
            v1_pool = ctx.enter_context(tc.tile_pool(name="v1", bufs=5))
            v2_pool = ctx.enter_context(tc.tile_pool(name="v2", bufs=5))
            v3_pool = ctx.enter_context(tc.tile_pool(name="v3", bufs=5))
            accd_pool = ctx.enter_context(tc.tile_pool(name="accd", bufs=3))
            accg_pool = ctx.enter_context(tc.tile_pool(name="accg", bufs=2))
            # PSUM: pa 1x2 banks + pb 2x2 banks + pc 2x1 bank = 8 banks
            pa_pool = ctx.enter_context(tc.tile_pool(name="pa", bufs=1, space="PSUM"))
            pb_pool = ctx.enter_context(tc.tile_pool(name="pb", bufs=2, space="PSUM"))
            pc_pool = ctx.enter_context(tc.tile_pool(name="pc", bufs=2, space="PSUM"))

            # stage-1 consts first so the prologue can start ASAP; the
            # bulky depthwise/project weights stream in behind them.
            w1s_sb = consts.tile([128, HID], f16)
            nc.sync.dma_start(w1s_sb[:], w1s)
            s1m_sb = consts.tile([128, NG], fp32)
            nc.sync.dma_start(s1m_sb[:], s1m)
            b1m_sb = consts.tile([128, NG], fp32)
            nc.sync.dma_start(b1m_sb[:], b1m)

            def late_consts():
                wdw_sb = consts.tile([128, NG * 9 * 128], f16)
                nc.sync.dma_start(wdw_sb[:], wdw)
                w3i_sb = consts.tile([128, NG * 64], f16)
                nc.sync.dma_start(w3i_sb[:], w3i)
                s2m_sb = consts.tile([128, NG], fp32)
                nc.sync.dma_start(s2m_sb[:], s2m)
                b2m_sb = consts.tile([128, NG], fp32)
                nc.sync.dma_start(b2m_sb[:], b2m)
                s3m_sb = consts.tile([64, 1], fp32)
                nc.sync.dma_start(s3m_sb[:], s3m)
                b3m_sb = consts.tile([64, 1], fp32)
                nc.sync.dma_start(b3m_sb[:], b3m)
                wdv_sb = consts.tile([128, NG * 9], fp32)
                nc.sync.dma_start(wdv_sb[:], wdv)
                return wdw_sb, w3i_sb, s2m_sb, b2m_sb, s3m_sb, b3m_sb, wdv_sb

            # persistent padded H1 (offset integer grid r1+1024), double
            # buffered by image parity; borders hold 1024 (= r1 of 0) so
            # the absorbed-offset bias correction is exact at edges too.
            h1p = [[h1p_pool.tile([128, PPIX], f16, tag=f"h1p{p}{g}",
                                  name=f"h1p{p}{g}") for g in range(NG)]
                   for p in range(2)]
            h1v = [[t[:].rearrange("p (h w) -> p h w", w=PW) for t in h1p[p]]
                   for p in range(2)]
            # only the 1-pixel border needs the 1024 fill (the interior is
            # overwritten every image); whole-tile memsets would serialize
            # ~17us on GpSimd before stage-1 can write.
            for p in range(2):
                for g in range(NG):
                    hv = h1v[p][g]
                    nc.gpsimd.memset(hv[:, 0:1, :], float(OFF))
                    nc.gpsimd.memset(hv[:, PH - 1:PH, :], float(OFF))
                    nc.gpsimd.memset(hv[:, 1:PH - 1, 0:1], float(OFF))
                    nc.gpsimd.memset(hv[:, 1:PH - 1, PW - 1:PW], float(OFF))
            h2t = [[h2_pool.tile([128, PIX], f16, tag=f"h2{p}{g}",
                                 name=f"h2{p}{g}") for g in range(NG)]
                   for p in range(2)]

            def dma_in(i, split=False):
                xhl_sb = x_pool.tile([128, PIX], f16, tag="xhl")
                if split:
                    # head slice first so the first stage-1 matmul can
                    # start without waiting for the full image
                    nc.sync.dma_start(xhl_sb[:, 0:2 * TW], xhl[i, :, 0:2 * TW])
                    nc.sync.dma_start(xhl_sb[:, 2 * TW:PIX],
                                      xhl[i, :, 2 * TW:PIX])
                else:
                    nc.sync.dma_start(xhl_sb[:], xhl[i, :, :])
                xp_sb = x_pool.tile([64, PIX], fp32, tag="xp")
                nc.sync.dma_start(xp_sb[:], xp[i, :, :])
                return xhl_sb, xp_sb

            def emit_s1(i, g, u, xhl_sb):
                p = i % 2
                nt = 2 if u < NU - 1 else NT - 2 * (NU - 1)
                w = TW * nt
                pa = pa_pool.tile([128, 1024], fp32)
                for j in range(nt):
                    t = 2 * u + j
                    nc.tensor.matmul(
                        pa[:, 512 * j:512 * j + TW],
                        w1s_sb[:, 128 * g:128 * (g + 1)],
                        xhl_sb[:, TW * t:TW * (t + 1)],
                        start=True, stop=True)
                pav = pa[:].rearrange("q (b c) -> q b c", c=512)[:, 0:nt, 0:TW]
                v = v1_pool.tile([128, 2 * TW], f16)
                nc.scalar.activation(v[:, 0:w], pav, IDENT,
                                     bias=b1m_sb[:, g:g + 1],
                                     scale=s1m_sb[:, g:g + 1])
                r0 = ROWS_PT * 2 * u + 1
                nc.vector.tensor_scalar(
                    h1v[p][g][:, r0:r0 + ROWS_PT * nt, 1:57], v[:, 0:w],
                    CLO, CHI, op0=mx, op1=mn)

            def emit_s2(i, g, u):
                p = i % 2
                nt = 2 if u < NU - 1 else NT - 2 * (NU - 1)
                w = TW * nt
                pb = pb_pool.tile([128, 1024], fp32)
                # tap-major: consecutive matmuls share lhsT
                for k, (dy, dx) in enumerate(TAPS):
                    lcol = 128 * (9 * g + k)
                    for j in range(nt):
                        t = 2 * u + j
                        r0 = ROWS_PT * t + 1
                        rhs = h1v[p][g][:, r0 + dy:r0 + dy + ROWS_PT,
                                        1 + dx:57 + dx]
                        nc.tensor.matmul(
                            pb[:, 512 * j:512 * j + TW],
                            wdw_sb[:, lcol:lcol + 128], rhs,
                            start=(k == 0), stop=(k == 8))
                pbv = pb[:].rearrange("q (b c) -> q b c", c=512)[:, 0:nt, 0:TW]
                v = v2_pool.tile([128, 2 * TW], f16)
                nc.scalar.activation(v[:, 0:w], pbv, IDENT,
                                     bias=b2m_sb[:, g:g + 1],
                                     scale=s2m_sb[:, g:g + 1])
                nc.vector.tensor_scalar(
                    h2t[p][g][:, 2 * TW * u:2 * TW * u + w], v[:, 0:w],
                    CLO, CHI, op0=mx, op1=mn)

            def emit_s2_vec(i, g, u, veng, acc_pool):
                # depthwise unit on DVE/GpSimd: 9 per-partition FMAs with
                # fp32 SBUF accumulation (exact: integer values), then the
                # usual ACT round + clamp.
                p = i % 2
                nt = 2 if u < NU - 1 else NT - 2 * (NU - 1)
                w = TW * nt
                rows = ROWS_PT * nt
                r0 = ROWS_PT * 2 * u + 1
                cur = acc_pool.tile([128, 2 * TW], fp32, tag="a", name="acc_a")
                nxt = acc_pool.tile([128, 2 * TW], fp32, tag="b", name="acc_b")
                for k, (dy, dx) in enumerate(TAPS):
                    win = h1v[p][g][:, r0 + dy:r0 + dy + rows, 1 + dx:57 + dx]
                    wap = wdv_sb[:, 9 * g + k:9 * g + k + 1]
                    if k == 0:
                        # first tap on ACT: w[c]*win with per-partition scale
                        nc.scalar.mul(cur[:, 0:w], win, wap)
                    else:
                        veng.scalar_tensor_tensor(nxt[:, 0:w], win, wap,
                                                  cur[:, 0:w],
                                                  op0=mult, op1=add)
                        cur, nxt = nxt, cur
                v = v2_pool.tile([128, 2 * TW], f16)
                nc.scalar.activation(v[:, 0:w], cur[:, 0:w], IDENT,
                                     bias=b2m_sb[:, g:g + 1],
                                     scale=s2m_sb[:, g:g + 1])
                veng.tensor_scalar(
                    h2t[p][g][:, 2 * TW * u:2 * TW * u + w], v[:, 0:w],
                    CLO, CHI, op0=mx, op1=mn)

            def emit_s3(i, t, xp_sb, o_sb):
                p = i % 2
                pc = pc_pool.tile([64, TW], fp32)
                for kc in range(NG):
                    nc.tensor.matmul(
                        pc[:], w3i_sb[:, 64 * kc:64 * (kc + 1)],
                        h2t[p][kc][:, TW * t:TW * (t + 1)],
                        start=(kc == 0), stop=(kc == NG - 1))
                v3 = v3_pool.tile([64, TW], f16, tag="v3", name="v3")
                nc.scalar.activation(v3[:], pc[:], IDENT,
                                     bias=b3m_sb[:, 0:1],
                                     scale=s3m_sb[:, 0:1])
                u3 = v3_pool.tile([64, TW], f16, tag="u3", name="u3")
                nc.vector.tensor_scalar(u3[:], v3[:], CLO, CHI,
                                        op0=mx, op1=mn)
                # out = (r3+1024)/255 + (x - 1024/255)
                nc.vector.scalar_tensor_tensor(
                    o_sb[:, TW * t:TW * (t + 1)], u3[:],
                    float(np.float32(1.0 / 255.0)),
                    xp_sb[:, TW * t:TW * (t + 1)],
                    op0=mult, op1=add)

            from collections import deque
            UNITS = [(g, u) for g in range(NG) for u in range(NU)]
            # last image: DVE units first so their chains drain early;
            # (2,1)/(2,2) last so only a short PE chain gates the tail
            UNITS_LAST = [(2, 0), (1, 0), (2, 3), (1, 1), (0, 0), (1, 2),
                          (0, 1), (1, 3), (0, 2), (0, 3), (2, 1), (2, 2)]
            # s3 tiles to pull after each slot of UNITS_LAST
            S3_PULL = {7: (0, 1), 10: (6,), 11: (2, 3)}
            bufs = {0: dma_in(0, split=True)}
            (wdw_sb, w3i_sb, s2m_sb, b2m_sb, s3m_sb, b3m_sb,
             wdv_sb) = late_consts()
            # group-pipelined prologue: only image 0 / group 0 runs bare;
            # the rest feeds through the s1 queue at 2 pulls/slot in iter 0.
            for u in range(NU):
                emit_s1(0, 0, u, bufs[0][0])
            s1q = deque((0, g, u) for g in (1, 2) for u in range(NU))

            def pull_s1():
                if s1q:
                    si, sg, su = s1q.popleft()
                    emit_s1(si, sg, su, bufs[si][0])

            for i in range(BPC):
                last = i + 1 >= BPC
                if not last:
                    bufs[i + 1] = dma_in(i + 1)
                    s1q.extend((i + 1, g, u) for (g, u) in UNITS)
                o_sb = o_pool.tile([64, PIX], fp32)
                s3q = list(range(NT))

                def do_s3(i, t, o_sb):
                    emit_s3(i, t, bufs[i][1], o_sb)
                    if t == 3:
                        # stream the first output half while the rest computes
                        nc.sync.dma_start(ys[i, :, 0:4 * TW],
                                          o_sb[:, 0:4 * TW])

                units_i = UNITS_LAST if last else UNITS
                for j, (g, u) in enumerate(units_i):
                    if (g, u) in GPS_S2:
                        emit_s2_vec(i, g, u, nc.gpsimd, accg_pool)
                    elif (g, u) in DVE_S2:
                        emit_s2_vec(i, g, u, nc.vector, accd_pool)
                    else:
                        emit_s2(i, g, u)
                    pull_s1()
                    if i == 0:
                        pull_s1()
                    if last:
                        for t in S3_PULL.get(j, ()):
                            if t in s3q:
                                s3q.remove(t)
                                do_s3(i, t, o_sb)
                for t in s3q:
                    do_s3(i, t, o_sb)
                nc.sync.dma_start(ys[i, :, 4 * TW:PIX], o_sb[:, 4 * TW:PIX])
                del bufs[i]

    nc.compile()
    return nc


def _prep_weights(inputs):
    inv1 = (inputs['g1'] / np.sqrt(inputs['v1'] + EPS)).astype(np.float32)
    beta1 = (inputs['b1'] - inputs['m1'] * inv1).astype(np.float32)
    inv2 = (inputs['g2'] / np.sqrt(inputs['v2'] + EPS)).astype(np.float32)
    beta2 = (inputs['b2'] - inputs['m2'] * inv2).astype(np.float32)
    inv3 = (inputs['g3'] / np.sqrt(inputs['v3'] + EPS)).astype(np.float32)
    beta3 = (inputs['b3'] - inputs['m3'] * inv3).astype(np.float32)

    w1q = _quant_w(inputs['w1'])[:, :, 0, 0]       # [384, 64]
    w2q = _quant_w(inputs['w2'])[:, 0, :, :]       # [384, 3, 3]
    w3q = _quant_w(inputs['w3'])[:, :, 0, 0]       # [64, 384]

    # integer weights (w*255 is an exact odd integer <= 255, fp16-exact)
    w1i = np.round(w1q * 255.0).astype(np.float32).T        # [64, 384]
    w1s = np.concatenate([w1i, w1i], axis=0).astype(np.float16)  # [128, 384]

    wdw_int = np.round(w2q * 255.0).astype(np.float32)      # [384, 3, 3]
    wdw = np.zeros((128, NG * 9 * 128), np.float16)
    for g in range(NG):
        ch = slice(128 * g, 128 * (g + 1))
        k = 0
        for dy in range(3):
            for dx in range(3):
                col = 128 * (9 * g + k)
                wdw[:, col:col + 128][np.arange(128), np.arange(128)] = \
                    wdw_int[ch, dy, dx].astype(np.float16)
                k += 1

    w3int = np.round(w3q * 255.0).astype(np.float32)        # [64, 384]
    w3i = np.zeros((128, NG * 64), np.float16)
    for kc in range(NG):
        w3i[:, 64 * kc:64 * (kc + 1)] = \
            w3int[:, 128 * kc:128 * (kc + 1)].T.astype(np.float16)

    # stage-1: t1 = pa*(inv1/6) + beta1*42.5 ; +1024 fp16 magic
    s1 = (inv1 / np.float32(6.0)).astype(np.float32)
    b1 = (beta1 * np.float32(42.5) + OFF).astype(np.float32)
    s1m = s1.reshape(NG, 128).T.copy()
    b1m = b1.reshape(NG, 128).T.copy()

    # stage-2: inputs carry +1024; absorb 1024*sum(w2) into the bias
    sumw2 = wdw_int.sum(axis=(1, 2)).astype(np.float32)     # [384]
    s2 = (inv2 / np.float32(255.0)).astype(np.float32)
    b2 = (beta2 * np.float32(42.5) + OFF - s2 * OFF * sumw2).astype(np.float32)
    s2m = s2.reshape(NG, 128).T.copy()
    b2m = b2.reshape(NG, 128).T.copy()

    # stage-3: inputs carry +1024; absorb 1024*sum(w3) into the bias
    sumw3 = w3int.sum(axis=1).astype(np.float32)            # [64]
    s3 = (np.float32(6.0) * inv3 / np.float32(255.0)).astype(np.float32)
    b3 = (beta3 * np.float32(255.0) + OFF - s3 * OFF * sumw3).astype(np.float32)
    s3m = s3.reshape(64, 1)
    b3m = b3.reshape(64, 1)

    # per-partition tap weights for vector-engine depthwise units
    wdv = np.zeros((128, NG * 9), np.float32)
    for g in range(NG):
        k = 0
        for dy in range(3):
            for dx in range(3):
                wdv[:, 9 * g + k] = wdw_int[128 * g:128 * (g + 1), dy, dx]
                k += 1

    return (w1s, np.ascontiguousarray(wdw), np.ascontiguousarray(w3i),
            np.ascontiguousarray(wdv),
            np.ascontiguousarray(s1m), np.ascontiguousarray(b1m),
            np.ascontiguousarray(s2m), np.ascontiguousarray(b2m),
            np.ascontiguousarray(s3m), np.ascontiguousarray(b3m))


def _make_inmaps(inputs):
    (w1s, wdw, w3i, wdv, s1m, b1m, s2m, b2m, s3m, b3m) = _prep_weights(inputs)
    x = np.asarray(inputs['x'], np.float32).reshape(B, C, PIX)
    x_hi = x.astype(np.float16)
    x_lo = (x - x_hi.astype(np.float32)).astype(np.float16)
    xhl = np.concatenate([x_hi, x_lo], axis=1)              # [B, 128, PIX]
    xp = (x - np.float32(OFF / np.float32(255.0))).astype(np.float32)

    in_maps = []
    for c in range(NCORES):
        sl = slice(BPC * c, BPC * (c + 1))
        in_maps.append({'xhl': np.ascontiguousarray(xhl[sl]),
                        'xp': np.ascontiguousarray(xp[sl]),
                        'w1s': w1s, 'wdw': wdw, 'w3i': w3i, 'wdv': wdv,
                        's1m': s1m, 'b1m': b1m, 's2m': s2m, 'b2m': b2m,
                        's3m': s3m, 'b3m': b3m})
    return in_maps


def kernel(**inputs):
    from concourse import bass_utils

    if 'nc' not in _cache:
        _cache['nc'] = _build_program()
    nc = _cache['nc']

    in_maps = _make_inmaps(inputs)
    res = bass_utils.run_bass_kernel_spmd(nc, in_maps, list(range(NCORES)))
    out = np.concatenate([res.results[c]['ys'] for c in range(NCORES)], axis=0)
    return out.reshape(B, C, H, W).astype(np.float32)


# revision 33
# speedup vs baseline: 1.0588x; 1.0588x over previous
"""Trainium2 Bass kernel for quantized InvertedResidual block (DoReFa fake-quant).

Strategy (v3, all-fp16 matmuls + software pipelining):
- Data-parallel: 32 images -> 4 per core across 8 NeuronCores.
- All matmuls fp16 (1 cycle/row on PE vs 4 for fp32):
  stage1: x split into fp16 hi+lo (22 mantissa bits), packed as K=128
          with duplicated integer weights [w1i; w1i] (w1i = w1q*255,
          exact in fp16) -> one matmul per (group, pixel tile).
  stage2: depthwise 3x3 = 9 diagonal matmuls, integer weights.
  stage3: 1x1 conv with integer weights w3q*255; activations are exact
          integers so products/accumulation are exact in PSUM.
- fp16 magic rounding: ACT computes scale*psum + (bias + 1024) in fp32;
  its fp16 output downcast rounds to the integer grid via RNE
  (v + 1024 in [1024, 2048) has ulp 1 in fp16). Activations are stored
  offset by +1024; the next stage's bias absorbs 1024*sum(weights)
  exactly. One DVE clamp (max 1024, min 1279) per tile finishes the
  quant. Residual uses x' = x - 1024/255 prepped on host.
- ACT/DVE ops cover two 448-pixel tiles at once (PSUM tiles span two
  banks, matmuls write at bank-aligned offsets 0/512) to amortize the
  ~370ns ACT init overhead.
- Software pipeline: image i+1's stage-1 units are interleaved into
  image i's stage-2 stream so the PE never stalls waiting for the
  (slower) ACT drain of stage-1 PSUM tiles; h1/h2 are double-buffered
  by image parity.
"""
import numpy as np

EPS = 1e-5
OFF = np.float32(1024.0)   # fp16 magic offset: [1024, 2048) has ulp 1

B, C, H, W = 32, 64, 56, 56
HID = 384
NCORES = 8
BPC = B // NCORES          # images per core
PIX = H * W                # 3136
PW = W + 2                 # 58
PH = H + 2
PPIX = PW * PH             # 3364
NT = 7                     # pixel tiles per image
TW = PIX // NT             # 448 = 8 rows x 56
ROWS_PT = H // NT          # 8
NG = HID // 128            # 3 channel groups
NU = (NT + 1) // 2         # 4 double-width units per (group, image)

_cache = {}


def _quant_w(w):
    # DoReFa weight fake-quant, computed with jax on CPU so tanh/round are
    # bitwise identical to the reference implementation.
    import jax
    import jax.numpy as jnp
    with jax.default_device(jax.devices('cpu')[0]):
        t = jnp.tanh(jnp.asarray(w, jnp.float32))
        m = jnp.max(jnp.abs(t), axis=(1, 2, 3), keepdims=True)
        wn = t / (2.0 * m) + 0.5
        q = 2.0 * jnp.round(wn * 255.0) / 255.0 - 1.0
        return np.asarray(q, np.float32)


def _build_program():
    import concourse.bass as bass
    import concourse.tile as tile
    from concourse import bacc, mybir

    fp32 = mybir.dt.float32
    f16 = mybir.dt.float16
    nc = bacc.Bacc("TRN2", target_bir_lowering=False, debug=False,
                   enable_asserts=False, num_devices=NCORES)

    xhl = nc.dram_tensor("xhl", [BPC, 128, PIX], f16, kind="ExternalInput").ap()
    xp = nc.dram_tensor("xp", [BPC, 64, PIX], fp32, kind="ExternalInput").ap()
    w1s = nc.dram_tensor("w1s", [128, HID], f16, kind="ExternalInput").ap()
    wdw = nc.dram_tensor("wdw", [128, NG * 9 * 128], f16, kind="ExternalInput").ap()
    w3i = nc.dram_tensor("w3i", [128, NG * 64], f16, kind="ExternalInput").ap()
    s1m = nc.dram_tensor("s1m", [128, NG], fp32, kind="ExternalInput").ap()
    b1m = nc.dram_tensor("b1m", [128, NG], fp32, kind="ExternalInput").ap()
    s2m = nc.dram_tensor("s2m", [128, NG], fp32, kind="ExternalInput").ap()
    b2m = nc.dram_tensor("b2m", [128, NG], fp32, kind="ExternalInput").ap()
    s3m = nc.dram_tensor("s3m", [64, 1], fp32, kind="ExternalInput").ap()
    b3m = nc.dram_tensor("b3m", [64, 1], fp32, kind="ExternalInput").ap()
    wdv = nc.dram_tensor("wdv", [128, NG * 9], fp32, kind="ExternalInput").ap()
    ys = nc.dram_tensor("ys", [BPC, 64, PIX], fp32, kind="ExternalOutput").ap()

    # stage-2 units computed on the vector engines instead of the PE
    # (depthwise tap = per-partition FMA via scalar_tensor_tensor)
    GPS_S2 = set()
    DVE_S2 = {(2, 0), (0, 1), (2, 3)}
    # PE stage-2 units whose tap-0 is seeded into PSUM by the ACT engine:
    # DISABLED — ACT->PSUM seeding gives wrong results on HW and the PSUM
    # write contention slows every matmul down.
    ACT_TAP0 = set()

    mx = mybir.AluOpType.max
    mn = mybir.AluOpType.min
    add = mybir.AluOpType.add
    mult = mybir.AluOpType.mult
    IDENT = mybir.ActivationFunctionType.Identity

    CLO = float(OFF)           # 1024.0
    CHI = float(OFF) + 255.0   # 1279.0
    TAPS = [(dy, dx) for dy in (-1, 0, 1) for dx in (-1, 0, 1)]

    with tile.TileContext(nc) as tc:
        from contextlib import ExitStack
        with ExitStack() as ctx:
            consts = ctx.enter_context(tc.tile_pool(name="consts", bufs=1))
            h1p_pool = ctx.enter_context(tc.tile_pool(name="h1p", bufs=1))
            h2_pool = ctx.enter_context(tc.tile_pool(name="h2", bufs=1))
            x_pool = ctx.enter_context(tc.tile_pool(name="x", bufs=2))
            o_pool = ctx.enter_context(tc.tile_pool(name="o", bufs=2))
            v1_pool = ctx.enter_context(tc.tile_pool(name="v1", bufs=5))
            v2_pool = ctx.enter_context(tc.tile_pool(name="v2", bufs=5))
            v3_pool = ctx.enter_context(tc.tile_pool(name="v3", bufs=5))
            accd_pool = ctx.enter_context(tc.tile_pool(name="accd", bufs=3))
            accg_pool = ctx.enter_context(tc.tile_pool(name="accg", bufs=2))
            # PSUM: pa 1x2 banks + pb 2x2 banks + pc 2x1 bank = 8 banks
            pa_pool = ctx.enter_context(tc.tile_pool(name="pa", bufs=1, space="PSUM"))
            pb_pool = ctx.enter_context(tc.tile_pool(name="pb", bufs=2, space="PSUM"))
            pc_pool = ctx.enter_context(tc.tile_pool(name="pc", bufs=2, space="PSUM"))

            # stage-1 consts first so the prologue can start ASAP; the
            # bulky depthwise/project weights stream in behind them.
            w1s_sb = consts.tile([128, HID], f16)
            nc.sync.dma_start(w1s_sb[:], w1s)
            s1m_sb = consts.tile([128, NG], fp32)
            nc.sync.dma_start(s1m_sb[:], s1m)
            b1m_sb = consts.tile([128, NG], fp32)
            nc.sync.dma_start(b1m_sb[:], b1m)

            def late_consts():
                wdw_sb = consts.tile([128, NG * 9 * 128], f16)
                nc.sync.dma_start(wdw_sb[:], wdw)
                w3i_sb = consts.tile([128, NG * 64], f16)
                nc.sync.dma_start(w3i_sb[:], w3i)
                s2m_sb = consts.tile([128, NG], fp32)
                nc.sync.dma_start(s2m_sb[:], s2m)
                b2m_sb = consts.tile([128, NG], fp32)
                nc.sync.dma_start(b2m_sb[:], b2m)
                s3m_sb = consts.tile([64, 1], fp32)
                nc.sync.dma_start(s3m_sb[:], s3m)
                b3m_sb = consts.tile([64, 1], fp32)
                nc.sync.dma_start(b3m_sb[:], b3m)
                wdv_sb = consts.tile([128, NG * 9], fp32)
                nc.sync.dma_start(wdv_sb[:], wdv)
                return wdw_sb, w3i_sb, s2m_sb, b2m_sb, s3m_sb, b3m_sb, wdv_sb

            # persistent padded H1 (offset integer grid r1+1024), double
            # buffered by image parity; borders hold 1024 (= r1 of 0) so
            # the absorbed-offset bias correction is exact at edges too.
            h1p = [[h1p_pool.tile([128, PPIX], f16, tag=f"h1p{p}{g}",
                                  name=f"h1p{p}{g}") for g in range(NG)]
                   for p in range(2)]
            h1v = [[t[:].rearrange("p (h w) -> p h w", w=PW) for t in h1p[p]]
                   for p in range(2)]
            # only the 1-pixel border needs the 1024 fill (the interior is
            # overwritten every image); whole-tile memsets would serialize
            # ~17us on GpSimd before stage-1 can write.
            for p in range(2):
                for g in range(NG):
                    hv = h1v[p][g]
                    nc.gpsimd.memset(hv[:, 0:1, :], float(OFF))
                    nc.gpsimd.memset(hv[:, PH - 1:PH, :], float(OFF))
                    nc.gpsimd.memset(hv[:, 1:PH - 1, 0:1], float(OFF))
                    nc.gpsimd.memset(hv[:, 1:PH - 1, PW - 1:PW], float(OFF))
            h2t = [[h2_pool.tile([128, PIX], f16, tag=f"h2{p}{g}",
                                 name=f"h2{p}{g}") for g in range(NG)]
                   for p in range(2)]

            def dma_in(i, split=False):
                xhl_sb = x_pool.tile([128, PIX], f16, tag="xhl")
                if split:
                    # head slice first so the first stage-1 matmul can
                    # start without waiting for the full image
                    nc.sync.dma_start(xhl_sb[:, 0:2 * TW], xhl[i, :, 0:2 * TW])
                    nc.sync.dma_start(xhl_sb[:, 2 * TW:PIX],
                                      xhl[i, :, 2 * TW:PIX])
                else:
                    nc.sync.dma_start(xhl_sb[:], xhl[i, :, :])
                xp_sb = x_pool.tile([64, PIX], fp32, tag="xp")
                nc.sync.dma_start(xp_sb[:], xp[i, :, :])
                return xhl_sb, xp_sb

            def emit_s1(i, g, u, xhl_sb):
                p = i % 2
                nt = 2 if u < NU - 1 else NT - 2 * (NU - 1)
                w = TW * nt
                pa = pa_pool.tile([128, 1024], fp32)
                for j in range(nt):
                    t = 2 * u + j
                    nc.tensor.matmul(
                        pa[:, 512 * j:512 * j + TW],
                        w1s_sb[:, 128 * g:128 * (g + 1)],
                        xhl_sb[:, TW * t:TW * (t + 1)],
                        start=True, stop=True)
                pav = pa[:].rearrange("q (b c) -> q b c", c=512)[:, 0:nt, 0:TW]
                v = v1_pool.tile([128, 2 * TW], f16)
                nc.scalar.activation(v[:, 0:w], pav, IDENT,
                                     bias=b1m_sb[:, g:g + 1],
                                     scale=s1m_sb[:, g:g + 1])
                r0 = ROWS_PT * 2 * u + 1
                nc.vector.tensor_scalar(
                    h1v[p][g][:, r0:r0 + ROWS_PT * nt, 1:57], v[:, 0:w],
                    CLO, CHI, op0=mx, op1=mn)

            def emit_s2(i, g, u):
                p = i % 2
                nt = 2 if u < NU - 1 else NT - 2 * (NU - 1)
                w = TW * nt
                pb = pb_pool.tile([128, 1024], fp32)
                # tap-major: consecutive matmuls share lhsT
                for k, (dy, dx) in enumerate(TAPS):
                    lcol = 128 * (9 * g + k)
                    for j in range(nt):
                        t = 2 * u + j
                        r0 = ROWS_PT * t + 1
                        rhs = h1v[p][g][:, r0 + dy:r0 + dy + ROWS_PT,
                                        1 + dx:57 + dx]
                        nc.tensor.matmul(
                            pb[:, 512 * j:512 * j + TW],
                            wdw_sb[:, lcol:lcol + 128], rhs,
                            start=(k == 0), stop=(k == 8))
                pbv = pb[:].rearrange("q (b c) -> q b c", c=512)[:, 0:nt, 0:TW]
                v = v2_pool.tile([128, 2 * TW], f16)
                nc.scalar.activation(v[:, 0:w], pbv, IDENT,
                                     bias=b2m_sb[:, g:g + 1],
                                     scale=s2m_sb[:, g:g + 1])
                nc.vector.tensor_scalar(
                    h2t[p][g][:, 2 * TW * u:2 * TW * u + w], v[:, 0:w],
                    CLO, CHI, op0=mx, op1=mn)

            def emit_s2_vec(i, g, u, veng, acc_pool):
                # depthwise unit on DVE/GpSimd: 9 per-partition FMAs with
                # fp32 SBUF accumulation (exact: integer values), then the
                # usual ACT round + clamp.
                p = i % 2
                nt = 2 if u < NU - 1 else NT - 2 * (NU - 1)
                w = TW * nt
                rows = ROWS_PT * nt
                r0 = ROWS_PT * 2 * u + 1
                cur = acc_pool.tile([128, 2 * TW], fp32, tag="a", name="acc_a")
                nxt = acc_pool.tile([128, 2 * TW], fp32, tag="b", name="acc_b")
                for k, (dy, dx) in enumerate(TAPS):
                    win = h1v[p][g][:, r0 + dy:r0 + dy + rows, 1 + dx:57 + dx]
                    wap = wdv_sb[:, 9 * g + k:9 * g + k + 1]
                    if k == 0:
                        # first tap on ACT: w[c]*win with per-partition scale
                        nc.scalar.mul(cur[:, 0:w], win, wap)
                    else:
                        veng.scalar_tensor_tensor(nxt[:, 0:w], win, wap,
                                                  cur[:, 0:w],
                                                  op0=mult, op1=add)
                        cur, nxt = nxt, cur
                v = v2_pool.tile([128, 2 * TW], f16)
                nc.scalar.activation(v[:, 0:w], cur[:, 0:w], IDENT,
                                     bias=b2m_sb[:, g:g + 1],
                                     scale=s2m_sb[:, g:g + 1])
                veng.tensor_scalar(
                    h2t[p][g][:, 2 * TW * u:2 * TW * u + w], v[:, 0:w],
                    CLO, CHI, op0=mx, op1=mn)

            def emit_s3(i, t, xp_sb, o_sb):
                p = i % 2
                pc = pc_pool.tile([64, TW], fp32)
                for kc in range(NG):
                    nc.tensor.matmul(
                        pc[:], w3i_sb[:, 64 * kc:64 * (kc + 1)],
                        h2t[p][kc][:, TW * t:TW * (t + 1)],
                        start=(kc == 0), stop=(kc == NG - 1))
                v3 = v3_pool.tile([64, TW], f16, tag="v3", name="v3")
                nc.scalar.activation(v3[:], pc[:], IDENT,
                                     bias=b3m_sb[:, 0:1],
                                     scale=s3m_sb[:, 0:1])
                u3 = v3_pool.tile([64, TW], f16, tag="u3", name="u3")
                nc.vector.tensor_scalar(u3[:], v3[:], CLO, CHI,
                                        op0=mx, op1=mn)
                # out = (r3+1024)/255 + (x - 1024/255)
                nc.vector.scalar_tensor_tensor(
                    o_sb[:, TW * t:TW * (t + 1)], u3[:],
                    float(np.float32(1.0 / 255.0)),
                    xp_sb[:, TW * t:TW * (t + 1)],
                    op0=mult, op1=add)

            UNITS = [(g, u) for g in range(NG) for u in range(NU)]
            bufs = {0: dma_in(0, split=True)}
            (wdw_sb, w3i_sb, s2m_sb, b2m_sb, s3m_sb, b3m_sb,
             wdv_sb) = late_consts()
            for (g, u) in UNITS:
                emit_s1(0, g, u, bufs[0][0])
            for i in range(BPC):
                last = i + 1 >= BPC
                if not last:
                    bufs[i + 1] = dma_in(i + 1)
                o_sb = o_pool.tile([64, PIX], fp32)
                # last image: keep the tail PE-only (no DVE blob at slot 11)
                dve_s2 = {(2, 0), (0, 1)} if last else DVE_S2
                s3q = list(range(NT))

                def do_s3(i, t, o_sb):
                    emit_s3(i, t, bufs[i][1], o_sb)
                    if t == 3:
                        # stream the first output half while the rest computes
                        nc.sync.dma_start(ys[i, :, 0:4 * TW],
                                          o_sb[:, 0:4 * TW])

                for j, (g, u) in enumerate(UNITS):
                    if (g, u) in GPS_S2:
                        emit_s2_vec(i, g, u, nc.gpsimd, accg_pool)
                    elif (g, u) in dve_s2:
                        emit_s2_vec(i, g, u, nc.vector, accd_pool)
                    else:
                        emit_s2(i, g, u)
                    if not last:
                        emit_s1(i + 1, g, u, bufs[i + 1][0])
                    elif j >= 9:
                        # last image: drain s3 early as h2 tiles complete
                        for t in (2 * (j - 9), 2 * (j - 9) + 1):
                            if t in s3q:
                                s3q.remove(t)
                                do_s3(i, t, o_sb)
                for t in s3q:
                    do_s3(i, t, o_sb)
                nc.sync.dma_start(ys[i, :, 4 * TW:PIX], o_sb[:, 4 * TW:PIX])
                del bufs[i]

    nc.compile()
    return nc


def _prep_weights(inputs):
    inv1 = (inputs['g1'] / np.sqrt(inputs['v1'] + EPS)).astype(np.float32)
    beta1 = (inputs['b1'] - inputs['m1'] * inv1).astype(np.float32)
    inv2 = (inputs['g2'] / np.sqrt(inputs['v2'] + EPS)).astype(np.float32)
    beta2 = (inputs['b2'] - inputs['m2'] * inv2).astype(np.float32)
    inv3 = (inputs['g3'] / np.sqrt(inputs['v3'] + EPS)).astype(np.float32)
    beta3 = (inputs['b3'] - inputs['m3'] * inv3).astype(np.float32)

    w1q = _quant_w(inputs['w1'])[:, :, 0, 0]       # [384, 64]
    w2q = _quant_w(inputs['w2'])[:, 0, :, :]       # [384, 3, 3]
    w3q = _quant_w(inputs['w3'])[:, :, 0, 0]       # [64, 384]

    # integer weights (w*255 is an exact odd integer <= 255, fp16-exact)
    w1i = np.round(w1q * 255.0).astype(np.float32).T        # [64, 384]
    w1s = np.concatenate([w1i, w1i], axis=0).astype(np.float16)  # [128, 384]

    wdw_int = np.round(w2q * 255.0).astype(np.float32)      # [384, 3, 3]
    wdw = np.zeros((128, NG * 9 * 128), np.float16)
    for g in range(NG):
        ch = slice(128 * g, 128 * (g + 1))
        k = 0
        for dy in range(3):
            for dx in range(3):
                col = 128 * (9 * g + k)
                wdw[:, col:col + 128][np.arange(128), np.arange(128)] = \
                    wdw_int[ch, dy, dx].astype(np.float16)
                k += 1

    w3int = np.round(w3q * 255.0).astype(np.float32)        # [64, 384]
    w3i = np.zeros((128, NG * 64), np.float16)
    for kc in range(NG):
        w3i[:, 64 * kc:64 * (kc + 1)] = \
            w3int[:, 128 * kc:128 * (kc + 1)].T.astype(np.float16)

    # stage-1: t1 = pa*(inv1/6) + beta1*42.5 ; +1024 fp16 magic
    s1 = (inv1 / np.float32(6.0)).astype(np.float32)
    b1 = (beta1 * np.float32(42.5) + OFF).astype(np.float32)
    s1m = s1.reshape(NG, 128).T.copy()
    b1m = b1.reshape(NG, 128).T.copy()

    # stage-2: inputs carry +1024; absorb 1024*sum(w2) into the bias
    sumw2 = wdw_int.sum(axis=(1, 2)).astype(np.float32)     # [384]
    s2 = (inv2 / np.float32(255.0)).astype(np.float32)
    b2 = (beta2 * np.float32(42.5) + OFF - s2 * OFF * sumw2).astype(np.float32)
    s2m = s2.reshape(NG, 128).T.copy()
    b2m = b2.reshape(NG, 128).T.copy()

    # stage-3: inputs carry +1024; absorb 1024*sum(w3) into the bias
    sumw3 = w3int.sum(axis=1).astype(np.float32)            # [64]
    s3 = (np.float32(6.0) * inv3 / np.float32(255.0)).astype(np.float32)
    b3 = (beta3 * np.float32(255.0) + OFF - s3 * OFF * sumw3).astype(np.float32)
    s3m = s3.reshape(64, 1)
    b3m = b3.reshape(64, 1)

    # per-partition tap weights for vector-engine depthwise units
    wdv = np.zeros((128, NG * 9), np.float32)
    for g in range(NG):
        k = 0
        for dy in range(3):
            for dx in range(3):
                wdv[:, 9 * g + k] = wdw_int[128 * g:128 * (g + 1), dy, dx]
                k += 1

    return (w1s, np.ascontiguousarray(wdw), np.ascontiguousarray(w3i),
            np.ascontiguousarray(wdv),
            np.ascontiguousarray(s1m), np.ascontiguousarray(b1m),
            np.ascontiguousarray(s2m), np.ascontiguousarray(b2m),
            np.ascontiguousarray(s3m), np.ascontiguousarray(b3m))


def _make_inmaps(inputs):
    (w1s, wdw, w3i, wdv, s1m, b1m, s2m, b2m, s3m, b3m) = _prep_weights(inputs)
    x = np.asarray(inputs['x'], np.float32).reshape(B, C, PIX)
    x_hi = x.astype(np.float16)
    x_lo = (x - x_hi.astype(np.float32)).astype(np.float16)
    xhl = np.concatenate([x_hi, x_lo], axis=1)              # [B, 128, PIX]
    xp = (x - np.float32(OFF / np.float32(255.0))).astype(np.float32)

    in_maps = []
    for c in range(NCORES):
        sl = slice(BPC * c, BPC * (c + 1))
        in_maps.append({'xhl': np.ascontiguousarray(xhl[sl]),
                        'xp': np.ascontiguousarray(xp[sl]),
                        'w1s': w1s, 'wdw': wdw, 'w3i': w3i, 'wdv': wdv,
                        's1m': s1m, 'b1m': b1m, 's2m': s2m, 'b2m': b2m,
                        's3m': s3m, 'b3m': b3m})
    return in_maps


def kernel(**inputs):
    from concourse import bass_utils

    if 'nc' not in _cache:
        _cache['nc'] = _build_program()
    nc = _cache['nc']

    in_maps = _make_inmaps(inputs)
    res = bass_utils.run_bass_kernel_spmd(nc, in_maps, list(range(NCORES)))
    out = np.concatenate([res.results[c]['ys'] for c in range(NCORES)], axis=0)
    return out.reshape(B, C, H, W).astype(np.float32)


# revision 36
# speedup vs baseline: 1.0782x; 1.0184x over previous
"""Trainium2 Bass kernel for quantized InvertedResidual block (DoReFa fake-quant).

Strategy (v3, all-fp16 matmuls + software pipelining):
- Data-parallel: 32 images -> 4 per core across 8 NeuronCores.
- All matmuls fp16 (1 cycle/row on PE vs 4 for fp32):
  stage1: x split into fp16 hi+lo (22 mantissa bits), packed as K=128
          with duplicated integer weights [w1i; w1i] (w1i = w1q*255,
          exact in fp16) -> one matmul per (group, pixel tile).
  stage2: depthwise 3x3 = 9 diagonal matmuls, integer weights.
  stage3: 1x1 conv with integer weights w3q*255; activations are exact
          integers so products/accumulation are exact in PSUM.
- fp16 magic rounding: ACT computes scale*psum + (bias + 1024) in fp32;
  its fp16 output downcast rounds to the integer grid via RNE
  (v + 1024 in [1024, 2048) has ulp 1 in fp16). Activations are stored
  offset by +1024; the next stage's bias absorbs 1024*sum(weights)
  exactly. One DVE clamp (max 1024, min 1279) per tile finishes the
  quant. Residual uses x' = x - 1024/255 prepped on host.
- ACT/DVE ops cover two 448-pixel tiles at once (PSUM tiles span two
  banks, matmuls write at bank-aligned offsets 0/512) to amortize the
  ~370ns ACT init overhead.
- Software pipeline: image i+1's stage-1 units are interleaved into
  image i's stage-2 stream so the PE never stalls waiting for the
  (slower) ACT drain of stage-1 PSUM tiles; h1/h2 are double-buffered
  by image parity.
"""
import numpy as np

EPS = 1e-5
OFF = np.float32(1024.0)   # fp16 magic offset: [1024, 2048) has ulp 1

B, C, H, W = 32, 64, 56, 56
HID = 384
NCORES = 8
BPC = B // NCORES          # images per core
PIX = H * W                # 3136
PW = W + 2                 # 58
PH = H + 2
PPIX = PW * PH             # 3364
NT = 7                     # pixel tiles per image
TW = PIX // NT             # 448 = 8 rows x 56
ROWS_PT = H // NT          # 8
NG = HID // 128            # 3 channel groups
NU = (NT + 1) // 2         # 4 double-width units per (group, image)

_cache = {}


def _quant_w(w):
    # DoReFa weight fake-quant, computed with jax on CPU so tanh/round are
    # bitwise identical to the reference implementation.
    import jax
    import jax.numpy as jnp
    with jax.default_device(jax.devices('cpu')[0]):
        t = jnp.tanh(jnp.asarray(w, jnp.float32))
        m = jnp.max(jnp.abs(t), axis=(1, 2, 3), keepdims=True)
        wn = t / (2.0 * m) + 0.5
        q = 2.0 * jnp.round(wn * 255.0) / 255.0 - 1.0
        return np.asarray(q, np.float32)


def _build_program():
    import concourse.bass as bass
    import concourse.tile as tile
    from concourse import bacc, mybir

    fp32 = mybir.dt.float32
    f16 = mybir.dt.float16
    nc = bacc.Bacc("TRN2", target_bir_lowering=False, debug=False,
                   enable_asserts=False, num_devices=NCORES)

    xhl = nc.dram_tensor("xhl", [BPC, 128, PIX], f16, kind="ExternalInput").ap()
    xp = nc.dram_tensor("xp", [BPC, 64, PIX], fp32, kind="ExternalInput").ap()
    w1s = nc.dram_tensor("w1s", [128, HID], f16, kind="ExternalInput").ap()
    wdw = nc.dram_tensor("wdw", [128, NG * 9 * 128], f16, kind="ExternalInput").ap()
    w3i = nc.dram_tensor("w3i", [128, NG * 64], f16, kind="ExternalInput").ap()
    s1m = nc.dram_tensor("s1m", [128, NG], fp32, kind="ExternalInput").ap()
    b1m = nc.dram_tensor("b1m", [128, NG], fp32, kind="ExternalInput").ap()
    s2m = nc.dram_tensor("s2m", [128, NG], fp32, kind="ExternalInput").ap()
    b2m = nc.dram_tensor("b2m", [128, NG], fp32, kind="ExternalInput").ap()
    s3m = nc.dram_tensor("s3m", [64, 1], fp32, kind="ExternalInput").ap()
    b3m = nc.dram_tensor("b3m", [64, 1], fp32, kind="ExternalInput").ap()
    wdv = nc.dram_tensor("wdv", [128, NG * 9], fp32, kind="ExternalInput").ap()
    ys = nc.dram_tensor("ys", [BPC, 64, PIX], fp32, kind="ExternalOutput").ap()

    # stage-2 units computed on the vector engines instead of the PE
    # (depthwise tap = per-partition FMA via scalar_tensor_tensor)
    GPS_S2 = set()
    DVE_S2 = {(2, 0), (0, 1), (2, 3)}
    # PE stage-2 units whose tap-0 is seeded into PSUM by the ACT engine:
    # DISABLED — ACT->PSUM seeding gives wrong results on HW and the PSUM
    # write contention slows every matmul down.
    ACT_TAP0 = set()

    mx = mybir.AluOpType.max
    mn = mybir.AluOpType.min
    add = mybir.AluOpType.add
    mult = mybir.AluOpType.mult
    IDENT = mybir.ActivationFunctionType.Identity

    CLO = float(OFF)           # 1024.0
    CHI = float(OFF) + 255.0   # 1279.0
    TAPS = [(dy, dx) for dy in (-1, 0, 1) for dx in (-1, 0, 1)]

    with tile.TileContext(nc) as tc:
        from contextlib import ExitStack
        with ExitStack() as ctx:
            consts = ctx.enter_context(tc.tile_pool(name="consts", bufs=1))
            h1p_pool = ctx.enter_context(tc.tile_pool(name="h1p", bufs=1))
            h2_pool = ctx.enter_context(tc.tile_pool(name="h2", bufs=1))
            x_pool = ctx.enter_context(tc.tile_pool(name="x", bufs=2))
            o_pool = ctx.enter_context(tc.tile_pool(name="o", bufs=2))
            v1_pool = ctx.enter_context(tc.tile_pool(name="v1", bufs=5))
            v2_pool = ctx.enter_context(tc.tile_pool(name="v2", bufs=5))
            v3_pool = ctx.enter_context(tc.tile_pool(name="v3", bufs=5))
            accd_pool = ctx.enter_context(tc.tile_pool(name="accd", bufs=3))
            accg_pool = ctx.enter_context(tc.tile_pool(name="accg", bufs=2))
            # PSUM: pa 1x2 banks + pb 2x2 banks + pc 2x1 bank = 8 banks
            pa_pool = ctx.enter_context(tc.tile_pool(name="pa", bufs=1, space="PSUM"))
            pb_pool = ctx.enter_context(tc.tile_pool(name="pb", bufs=2, space="PSUM"))
            pc_pool = ctx.enter_context(tc.tile_pool(name="pc", bufs=2, space="PSUM"))

            # stage-1 consts first so the prologue can start ASAP; the
            # bulky depthwise/project weights stream in behind them.
            w1s_sb = consts.tile([128, HID], f16)
            nc.sync.dma_start(w1s_sb[:], w1s)
            s1m_sb = consts.tile([128, NG], fp32)
            nc.sync.dma_start(s1m_sb[:], s1m)
            b1m_sb = consts.tile([128, NG], fp32)
            nc.sync.dma_start(b1m_sb[:], b1m)

            def late_consts():
                wdw_sb = consts.tile([128, NG * 9 * 128], f16)
                nc.sync.dma_start(wdw_sb[:], wdw)
                w3i_sb = consts.tile([128, NG * 64], f16)
                nc.sync.dma_start(w3i_sb[:], w3i)
                s2m_sb = consts.tile([128, NG], fp32)
                nc.sync.dma_start(s2m_sb[:], s2m)
                b2m_sb = consts.tile([128, NG], fp32)
                nc.sync.dma_start(b2m_sb[:], b2m)
                s3m_sb = consts.tile([64, 1], fp32)
                nc.sync.dma_start(s3m_sb[:], s3m)
                b3m_sb = consts.tile([64, 1], fp32)
                nc.sync.dma_start(b3m_sb[:], b3m)
                wdv_sb = consts.tile([128, NG * 9], fp32)
                nc.sync.dma_start(wdv_sb[:], wdv)
                return wdw_sb, w3i_sb, s2m_sb, b2m_sb, s3m_sb, b3m_sb, wdv_sb

            # persistent padded H1 (offset integer grid r1+1024), double
            # buffered by image parity; borders hold 1024 (= r1 of 0) so
            # the absorbed-offset bias correction is exact at edges too.
            h1p = [[h1p_pool.tile([128, PPIX], f16, tag=f"h1p{p}{g}",
                                  name=f"h1p{p}{g}") for g in range(NG)]
                   for p in range(2)]
            h1v = [[t[:].rearrange("p (h w) -> p h w", w=PW) for t in h1p[p]]
                   for p in range(2)]
            # only the 1-pixel border needs the 1024 fill (the interior is
            # overwritten every image); whole-tile memsets would serialize
            # ~17us on GpSimd before stage-1 can write.
            for p in range(2):
                for g in range(NG):
                    hv = h1v[p][g]
                    nc.gpsimd.memset(hv[:, 0:1, :], float(OFF))
                    nc.gpsimd.memset(hv[:, PH - 1:PH, :], float(OFF))
                    nc.gpsimd.memset(hv[:, 1:PH - 1, 0:1], float(OFF))
                    nc.gpsimd.memset(hv[:, 1:PH - 1, PW - 1:PW], float(OFF))
            h2t = [[h2_pool.tile([128, PIX], f16, tag=f"h2{p}{g}",
                                 name=f"h2{p}{g}") for g in range(NG)]
                   for p in range(2)]

            def dma_in(i, split=False, defer_xp=False):
                xhl_sb = x_pool.tile([128, PIX], f16, tag="xhl")
                if split:
                    # head slice first so the first stage-1 matmul can
                    # start without waiting for the full image
                    nc.sync.dma_start(xhl_sb[:, 0:2 * TW], xhl[i, :, 0:2 * TW])
                    nc.sync.dma_start(xhl_sb[:, 2 * TW:PIX],
                                      xhl[i, :, 2 * TW:PIX])
                else:
                    nc.sync.dma_start(xhl_sb[:], xhl[i, :, :])
                xp_sb = x_pool.tile([64, PIX], fp32, tag="xp")
                if not defer_xp:
                    nc.sync.dma_start(xp_sb[:], xp[i, :, :])
                return xhl_sb, xp_sb

            def emit_s1(i, g, u, xhl_sb):
                p = i % 2
                nt = 2 if u < NU - 1 else NT - 2 * (NU - 1)
                w = TW * nt
                pa = pa_pool.tile([128, 1024], fp32)
                for j in range(nt):
                    t = 2 * u + j
                    nc.tensor.matmul(
                        pa[:, 512 * j:512 * j + TW],
                        w1s_sb[:, 128 * g:128 * (g + 1)],
                        xhl_sb[:, TW * t:TW * (t + 1)],
                        start=True, stop=True)
                pav = pa[:].rearrange("q (b c) -> q b c", c=512)[:, 0:nt, 0:TW]
                v = v1_pool.tile([128, 2 * TW], f16)
                nc.scalar.activation(v[:, 0:w], pav, IDENT,
                                     bias=b1m_sb[:, g:g + 1],
                                     scale=s1m_sb[:, g:g + 1])
                r0 = ROWS_PT * 2 * u + 1
                nc.vector.tensor_scalar(
                    h1v[p][g][:, r0:r0 + ROWS_PT * nt, 1:57], v[:, 0:w],
                    CLO, CHI, op0=mx, op1=mn)

            def emit_s2(i, g, u):
                p = i % 2
                nt = 2 if u < NU - 1 else NT - 2 * (NU - 1)
                w = TW * nt
                pb = pb_pool.tile([128, 1024], fp32)
                # tap-major: consecutive matmuls share lhsT
                for k, (dy, dx) in enumerate(TAPS):
                    lcol = 128 * (9 * g + k)
                    for j in range(nt):
                        t = 2 * u + j
                        r0 = ROWS_PT * t + 1
                        rhs = h1v[p][g][:, r0 + dy:r0 + dy + ROWS_PT,
                                        1 + dx:57 + dx]
                        nc.tensor.matmul(
                            pb[:, 512 * j:512 * j + TW],
                            wdw_sb[:, lcol:lcol + 128], rhs,
                            start=(k == 0), stop=(k == 8))
                pbv = pb[:].rearrange("q (b c) -> q b c", c=512)[:, 0:nt, 0:TW]
                v = v2_pool.tile([128, 2 * TW], f16)
                nc.scalar.activation(v[:, 0:w], pbv, IDENT,
                                     bias=b2m_sb[:, g:g + 1],
                                     scale=s2m_sb[:, g:g + 1])
                nc.vector.tensor_scalar(
                    h2t[p][g][:, 2 * TW * u:2 * TW * u + w], v[:, 0:w],
                    CLO, CHI, op0=mx, op1=mn)

            def emit_s2_vec(i, g, u, veng, acc_pool):
                # depthwise unit on DVE/GpSimd: 9 per-partition FMAs with
                # fp32 SBUF accumulation (exact: integer values), then the
                # usual ACT round + clamp.
                p = i % 2
                nt = 2 if u < NU - 1 else NT - 2 * (NU - 1)
                w = TW * nt
                rows = ROWS_PT * nt
                r0 = ROWS_PT * 2 * u + 1
                cur = acc_pool.tile([128, 2 * TW], fp32, tag="a", name="acc_a")
                nxt = acc_pool.tile([128, 2 * TW], fp32, tag="b", name="acc_b")
                for k, (dy, dx) in enumerate(TAPS):
                    win = h1v[p][g][:, r0 + dy:r0 + dy + rows, 1 + dx:57 + dx]
                    wap = wdv_sb[:, 9 * g + k:9 * g + k + 1]
                    if k == 0:
                        # first tap on ACT: w[c]*win with per-partition scale
                        nc.scalar.mul(cur[:, 0:w], win, wap)
                    else:
                        veng.scalar_tensor_tensor(nxt[:, 0:w], win, wap,
                                                  cur[:, 0:w],
                                                  op0=mult, op1=add)
                        cur, nxt = nxt, cur
                v = v2_pool.tile([128, 2 * TW], f16)
                nc.scalar.activation(v[:, 0:w], cur[:, 0:w], IDENT,
                                     bias=b2m_sb[:, g:g + 1],
                                     scale=s2m_sb[:, g:g + 1])
                veng.tensor_scalar(
                    h2t[p][g][:, 2 * TW * u:2 * TW * u + w], v[:, 0:w],
                    CLO, CHI, op0=mx, op1=mn)

            def emit_s3(i, t, xp_sb, o_sb):
                p = i % 2
                pc = pc_pool.tile([64, TW], fp32)
                for kc in range(NG):
                    nc.tensor.matmul(
                        pc[:], w3i_sb[:, 64 * kc:64 * (kc + 1)],
                        h2t[p][kc][:, TW * t:TW * (t + 1)],
                        start=(kc == 0), stop=(kc == NG - 1))
                v3 = v3_pool.tile([64, TW], f16, tag="v3", name="v3")
                nc.scalar.activation(v3[:], pc[:], IDENT,
                                     bias=b3m_sb[:, 0:1],
                                     scale=s3m_sb[:, 0:1])
                u3 = v3_pool.tile([64, TW], f16, tag="u3", name="u3")
                nc.vector.tensor_scalar(u3[:], v3[:], CLO, CHI,
                                        op0=mx, op1=mn)
                # out = (r3+1024)/255 + (x - 1024/255)
                nc.vector.scalar_tensor_tensor(
                    o_sb[:, TW * t:TW * (t + 1)], u3[:],
                    float(np.float32(1.0 / 255.0)),
                    xp_sb[:, TW * t:TW * (t + 1)],
                    op0=mult, op1=add)

            UNITS = [(g, u) for g in range(NG) for u in range(NU)]
            # image-0 xp is deferred behind the stage-2 weights (wdw must
            # land before the first s2 matmul ~13us in; xp isn't read
            # until stage 3 of image 0)
            bufs = {0: dma_in(0, split=True, defer_xp=True)}
            (wdw_sb, w3i_sb, s2m_sb, b2m_sb, s3m_sb, b3m_sb,
             wdv_sb) = late_consts()
            nc.sync.dma_start(bufs[0][1][:], xp[0, :, :])
            for (g, u) in UNITS:
                emit_s1(0, g, u, bufs[0][0])
            for i in range(BPC):
                last = i + 1 >= BPC
                if not last:
                    bufs[i + 1] = dma_in(i + 1)
                o_sb = o_pool.tile([64, PIX], fp32)
                dve_s2 = DVE_S2
                s3q = list(range(NT))

                def do_s3(i, t, o_sb):
                    emit_s3(i, t, bufs[i][1], o_sb)
                    if t == 3:
                        # stream the first output half while the rest computes
                        nc.sync.dma_start(ys[i, :, 0:4 * TW],
                                          o_sb[:, 0:4 * TW])

                for j, (g, u) in enumerate(UNITS):
                    if (g, u) in GPS_S2:
                        emit_s2_vec(i, g, u, nc.gpsimd, accg_pool)
                    elif (g, u) in dve_s2:
                        emit_s2_vec(i, g, u, nc.vector, accd_pool)
                    else:
                        emit_s2(i, g, u)
                    if not last:
                        emit_s1(i + 1, g, u, bufs[i + 1][0])
                    elif j >= 9:
                        # last image: drain s3 early as h2 tiles complete
                        for t in (2 * (j - 9), 2 * (j - 9) + 1):
                            if t in s3q:
                                s3q.remove(t)
                                do_s3(i, t, o_sb)
                for t in s3q:
                    do_s3(i, t, o_sb)
                nc.sync.dma_start(ys[i, :, 4 * TW:PIX], o_sb[:, 4 * TW:PIX])
                del bufs[i]

    nc.compile()
    return nc


def _prep_weights(inputs):
    inv1 = (inputs['g1'] / np.sqrt(inputs['v1'] + EPS)).astype(np.float32)
    beta1 = (inputs['b1'] - inputs['m1'] * inv1).astype(np.float32)
    inv2 = (inputs['g2'] / np.sqrt(inputs['v2'] + EPS)).astype(np.float32)
    beta2 = (inputs['b2'] - inputs['m2'] * inv2).astype(np.float32)
    inv3 = (inputs['g3'] / np.sqrt(inputs['v3'] + EPS)).astype(np.float32)
    beta3 = (inputs['b3'] - inputs['m3'] * inv3).astype(np.float32)

    w1q = _quant_w(inputs['w1'])[:, :, 0, 0]       # [384, 64]
    w2q = _quant_w(inputs['w2'])[:, 0, :, :]       # [384, 3, 3]
    w3q = _quant_w(inputs['w3'])[:, :, 0, 0]       # [64, 384]

    # integer weights (w*255 is an exact odd integer <= 255, fp16-exact)
    w1i = np.round(w1q * 255.0).astype(np.float32).T        # [64, 384]
    w1s = np.concatenate([w1i, w1i], axis=0).astype(np.float16)  # [128, 384]

    wdw_int = np.round(w2q * 255.0).astype(np.float32)      # [384, 3, 3]
    wdw = np.zeros((128, NG * 9 * 128), np.float16)
    for g in range(NG):
        ch = slice(128 * g, 128 * (g + 1))
        k = 0
        for dy in range(3):
            for dx in range(3):
                col = 128 * (9 * g + k)
                wdw[:, col:col + 128][np.arange(128), np.arange(128)] = \
                    wdw_int[ch, dy, dx].astype(np.float16)
                k += 1

    w3int = np.round(w3q * 255.0).astype(np.float32)        # [64, 384]
    w3i = np.zeros((128, NG * 64), np.float16)
    for kc in range(NG):
        w3i[:, 64 * kc:64 * (kc + 1)] = \
            w3int[:, 128 * kc:128 * (kc + 1)].T.astype(np.float16)

    # stage-1: t1 = pa*(inv1/6) + beta1*42.5 ; +1024 fp16 magic
    s1 = (inv1 / np.float32(6.0)).astype(np.float32)
    b1 = (beta1 * np.float32(42.5) + OFF).astype(np.float32)
    s1m = s1.reshape(NG, 128).T.copy()
    b1m = b1.reshape(NG, 128).T.copy()

    # stage-2: inputs carry +1024; absorb 1024*sum(w2) into the bias
    sumw2 = wdw_int.sum(axis=(1, 2)).astype(np.float32)     # [384]
    s2 = (inv2 / np.float32(255.0)).astype(np.float32)
    b2 = (beta2 * np.float32(42.5) + OFF - s2 * OFF * sumw2).astype(np.float32)
    s2m = s2.reshape(NG, 128).T.copy()
    b2m = b2.reshape(NG, 128).T.copy()

    # stage-3: inputs carry +1024; absorb 1024*sum(w3) into the bias
    sumw3 = w3int.sum(axis=1).astype(np.float32)            # [64]
    s3 = (np.float32(6.0) * inv3 / np.float32(255.0)).astype(np.float32)
    b3 = (beta3 * np.float32(255.0) + OFF - s3 * OFF * sumw3).astype(np.float32)
    s3m = s3.reshape(64, 1)
    b3m = b3.reshape(64, 1)

    # per-partition tap weights for vector-engine depthwise units
    wdv = np.zeros((128, NG * 9), np.float32)
    for g in range(NG):
        k = 0
        for dy in range(3):
            for dx in range(3):
                wdv[:, 9 * g + k] = wdw_int[128 * g:128 * (g + 1), dy, dx]
                k += 1

    return (w1s, np.ascontiguousarray(wdw), np.ascontiguousarray(w3i),
            np.ascontiguousarray(wdv),
            np.ascontiguousarray(s1m), np.ascontiguousarray(b1m),
            np.ascontiguousarray(s2m), np.ascontiguousarray(b2m),
            np.ascontiguousarray(s3m), np.ascontiguousarray(b3m))


def _make_inmaps(inputs):
    (w1s, wdw, w3i, wdv, s1m, b1m, s2m, b2m, s3m, b3m) = _prep_weights(inputs)
    x = np.asarray(inputs['x'], np.float32).reshape(B, C, PIX)
    x_hi = x.astype(np.float16)
    x_lo = (x - x_hi.astype(np.float32)).astype(np.float16)
    xhl = np.concatenate([x_hi, x_lo], axis=1)              # [B, 128, PIX]
    xp = (x - np.float32(OFF / np.float32(255.0))).astype(np.float32)

    in_maps = []
    for c in range(NCORES):
        sl = slice(BPC * c, BPC * (c + 1))
        in_maps.append({'xhl': np.ascontiguousarray(xhl[sl]),
                        'xp': np.ascontiguousarray(xp[sl]),
                        'w1s': w1s, 'wdw': wdw, 'w3i': w3i, 'wdv': wdv,
                        's1m': s1m, 'b1m': b1m, 's2m': s2m, 'b2m': b2m,
                        's3m': s3m, 'b3m': b3m})
    return in_maps


def kernel(**inputs):
    from concourse import bass_utils

    if 'nc' not in _cache:
        _cache['nc'] = _build_program()
    nc = _cache['nc']

    in_maps = _make_inmaps(inputs)
    res = bass_utils.run_bass_kernel_spmd(nc, in_maps, list(range(NCORES)))
    out = np.concatenate([res.results[c]['ys'] for c in range(NCORES)], axis=0)
    return out.reshape(B, C, H, W).astype(np.float32)


# revision 43
# speedup vs baseline: 1.0877x; 1.0087x over previous
"""Trainium2 Bass kernel for quantized InvertedResidual block (DoReFa fake-quant).

Strategy (v3, all-fp16 matmuls + software pipelining):
- Data-parallel: 32 images -> 4 per core across 8 NeuronCores.
- All matmuls fp16 (1 cycle/row on PE vs 4 for fp32):
  stage1: x split into fp16 hi+lo (22 mantissa bits), packed as K=128
          with duplicated integer weights [w1i; w1i] (w1i = w1q*255,
          exact in fp16) -> one matmul per (group, pixel tile).
  stage2: depthwise 3x3 = 9 diagonal matmuls, integer weights.
  stage3: 1x1 conv with integer weights w3q*255; activations are exact
          integers so products/accumulation are exact in PSUM.
- fp16 magic rounding: ACT computes scale*psum + (bias + 1024) in fp32;
  its fp16 output downcast rounds to the integer grid via RNE
  (v + 1024 in [1024, 2048) has ulp 1 in fp16). Activations are stored
  offset by +1024; the next stage's bias absorbs 1024*sum(weights)
  exactly. One DVE clamp (max 1024, min 1279) per tile finishes the
  quant. Residual uses x' = x - 1024/255 prepped on host.
- ACT/DVE ops cover two 448-pixel tiles at once (PSUM tiles span two
  banks, matmuls write at bank-aligned offsets 0/512) to amortize the
  ~370ns ACT init overhead.
- Software pipeline: image i+1's stage-1 units are interleaved into
  image i's stage-2 stream so the PE never stalls waiting for the
  (slower) ACT drain of stage-1 PSUM tiles; h1/h2 are double-buffered
  by image parity.
"""
import numpy as np

EPS = 1e-5
OFF = np.float32(1024.0)   # fp16 magic offset: [1024, 2048) has ulp 1

B, C, H, W = 32, 64, 56, 56
HID = 384
NCORES = 8
BPC = B // NCORES          # images per core
PIX = H * W                # 3136
PW = W + 2                 # 58
PH = H + 2
PPIX = PW * PH             # 3364
NT = 7                     # pixel tiles per image
TW = PIX // NT             # 448 = 8 rows x 56
ROWS_PT = H // NT          # 8
NG = HID // 128            # 3 channel groups
NU = (NT + 1) // 2         # 4 double-width units per (group, image)

_cache = {}


def _quant_w(w):
    # DoReFa weight fake-quant, computed with jax on CPU so tanh/round are
    # bitwise identical to the reference implementation.
    import jax
    import jax.numpy as jnp
    with jax.default_device(jax.devices('cpu')[0]):
        t = jnp.tanh(jnp.asarray(w, jnp.float32))
        m = jnp.max(jnp.abs(t), axis=(1, 2, 3), keepdims=True)
        wn = t / (2.0 * m) + 0.5
        q = 2.0 * jnp.round(wn * 255.0) / 255.0 - 1.0
        return np.asarray(q, np.float32)


def _build_program():
    import concourse.bass as bass
    import concourse.tile as tile
    from concourse import bacc, mybir

    fp32 = mybir.dt.float32
    f16 = mybir.dt.float16
    nc = bacc.Bacc("TRN2", target_bir_lowering=False, debug=False,
                   enable_asserts=False, num_devices=NCORES)

    xhl = nc.dram_tensor("xhl", [BPC, 128, PIX], f16, kind="ExternalInput").ap()
    xp = nc.dram_tensor("xp", [BPC, 64, PIX], fp32, kind="ExternalInput").ap()
    w1s = nc.dram_tensor("w1s", [128, HID], f16, kind="ExternalInput").ap()
    wdw = nc.dram_tensor("wdw", [128, NG * 9 * 128], f16, kind="ExternalInput").ap()
    w3i = nc.dram_tensor("w3i", [128, NG * 64], f16, kind="ExternalInput").ap()
    sb1 = nc.dram_tensor("sb1", [128, 2 * NG], fp32, kind="ExternalInput").ap()
    s2m = nc.dram_tensor("s2m", [128, NG], fp32, kind="ExternalInput").ap()
    b2m = nc.dram_tensor("b2m", [128, NG], fp32, kind="ExternalInput").ap()
    s3m = nc.dram_tensor("s3m", [64, 1], fp32, kind="ExternalInput").ap()
    b3m = nc.dram_tensor("b3m", [64, 1], fp32, kind="ExternalInput").ap()
    wdv = nc.dram_tensor("wdv", [128, NG * 9], fp32, kind="ExternalInput").ap()
    ys = nc.dram_tensor("ys", [BPC, 64, PIX], fp32, kind="ExternalOutput").ap()

    # stage-2 units computed on the vector engines instead of the PE
    # (depthwise tap = per-partition FMA via scalar_tensor_tensor)
    GPS_S2 = set()
    DVE_S2 = {(2, 0), (0, 1), (2, 3)}
    # PE stage-2 units whose tap-0 is seeded into PSUM by the ACT engine:
    # DISABLED — ACT->PSUM seeding gives wrong results on HW and the PSUM
    # write contention slows every matmul down.
    ACT_TAP0 = set()

    mx = mybir.AluOpType.max
    mn = mybir.AluOpType.min
    add = mybir.AluOpType.add
    mult = mybir.AluOpType.mult
    IDENT = mybir.ActivationFunctionType.Identity

    CLO = float(OFF)           # 1024.0
    CHI = float(OFF) + 255.0   # 1279.0
    TAPS = [(dy, dx) for dy in (-1, 0, 1) for dx in (-1, 0, 1)]

    with tile.TileContext(nc) as tc:
        from contextlib import ExitStack
        with ExitStack() as ctx:
            consts = ctx.enter_context(tc.tile_pool(name="consts", bufs=1))
            h1p_pool = ctx.enter_context(tc.tile_pool(name="h1p", bufs=1))
            h2_pool = ctx.enter_context(tc.tile_pool(name="h2", bufs=1))
            x_pool = ctx.enter_context(tc.tile_pool(name="x", bufs=2))
            o_pool = ctx.enter_context(tc.tile_pool(name="o", bufs=2))
            v1_pool = ctx.enter_context(tc.tile_pool(name="v1", bufs=5))
            v2_pool = ctx.enter_context(tc.tile_pool(name="v2", bufs=5))
            v3_pool = ctx.enter_context(tc.tile_pool(name="v3", bufs=5))
            accd_pool = ctx.enter_context(tc.tile_pool(name="accd", bufs=3))
            accg_pool = ctx.enter_context(tc.tile_pool(name="accg", bufs=2))
            # PSUM: pa 1x2 banks + pb 2x2 banks + pc 2x1 bank = 8 banks
            pa_pool = ctx.enter_context(tc.tile_pool(name="pa", bufs=1, space="PSUM"))
            pb_pool = ctx.enter_context(tc.tile_pool(name="pb", bufs=2, space="PSUM"))
            pc_pool = ctx.enter_context(tc.tile_pool(name="pc", bufs=2, space="PSUM"))

            # stage-1 consts first so the prologue can start ASAP; the
            # bulky depthwise/project weights stream in behind them.
            w1s_sb = consts.tile([128, HID], f16)
            nc.sync.dma_start(w1s_sb[:], w1s)
            sb1_sb = consts.tile([128, 2 * NG], fp32)
            nc.sync.dma_start(sb1_sb[:], sb1)

            def late_consts():
                wdw_sb = consts.tile([128, NG * 9 * 128], f16)
                nc.sync.dma_start(wdw_sb[:], wdw)
                w3i_sb = consts.tile([128, NG * 64], f16)
                nc.sync.dma_start(w3i_sb[:], w3i)
                s2m_sb = consts.tile([128, NG], fp32)
                nc.sync.dma_start(s2m_sb[:], s2m)
                b2m_sb = consts.tile([128, NG], fp32)
                nc.sync.dma_start(b2m_sb[:], b2m)
                s3m_sb = consts.tile([64, 1], fp32)
                nc.sync.dma_start(s3m_sb[:], s3m)
                b3m_sb = consts.tile([64, 1], fp32)
                nc.sync.dma_start(b3m_sb[:], b3m)
                wdv_sb = consts.tile([128, NG * 9], fp32)
                nc.sync.dma_start(wdv_sb[:], wdv)
                return wdw_sb, w3i_sb, s2m_sb, b2m_sb, s3m_sb, b3m_sb, wdv_sb

            # persistent padded H1 (offset integer grid r1+1024), double
            # buffered by image parity; borders hold 1024 (= r1 of 0) so
            # the absorbed-offset bias correction is exact at edges too.
            h1p = [[h1p_pool.tile([128, PPIX], f16, tag=f"h1p{p}{g}",
                                  name=f"h1p{p}{g}") for g in range(NG)]
                   for p in range(2)]
            h1v = [[t[:].rearrange("p (h w) -> p h w", w=PW) for t in h1p[p]]
                   for p in range(2)]
            # only the 1-pixel border needs the 1024 fill (the interior is
            # overwritten every image); whole-tile memsets would serialize
            # ~17us on GpSimd before stage-1 can write.
            for p in range(2):
                for g in range(NG):
                    hv = h1v[p][g]
                    nc.gpsimd.memset(hv[:, 0:1, :], float(OFF))
                    nc.gpsimd.memset(hv[:, PH - 1:PH, :], float(OFF))
                    nc.gpsimd.memset(hv[:, 1:PH - 1, 0:1], float(OFF))
                    nc.gpsimd.memset(hv[:, 1:PH - 1, PW - 1:PW], float(OFF))
            h2t = [[h2_pool.tile([128, PIX], f16, tag=f"h2{p}{g}",
                                 name=f"h2{p}{g}") for g in range(NG)]
                   for p in range(2)]

            def dma_in(i, split=False, defer_xp=False):
                xhl_sb = x_pool.tile([128, PIX], f16, tag="xhl")
                if split:
                    # head slice first so the first stage-1 matmul can
                    # start without waiting for the full image
                    nc.sync.dma_start(xhl_sb[:, 0:2 * TW], xhl[i, :, 0:2 * TW])
                    nc.sync.dma_start(xhl_sb[:, 2 * TW:PIX],
                                      xhl[i, :, 2 * TW:PIX])
                else:
                    nc.sync.dma_start(xhl_sb[:], xhl[i, :, :])
                xp_sb = x_pool.tile([64, PIX], fp32, tag="xp")
                if not defer_xp:
                    nc.sync.dma_start(xp_sb[:], xp[i, :, :])
                return xhl_sb, xp_sb

            def emit_s1(i, g, u, xhl_sb):
                p = i % 2
                nt = 2 if u < NU - 1 else NT - 2 * (NU - 1)
                w = TW * nt
                pa = pa_pool.tile([128, 1024], fp32)
                for j in range(nt):
                    t = 2 * u + j
                    nc.tensor.matmul(
                        pa[:, 512 * j:512 * j + TW],
                        w1s_sb[:, 128 * g:128 * (g + 1)],
                        xhl_sb[:, TW * t:TW * (t + 1)],
                        start=True, stop=True)
                pav = pa[:].rearrange("q (b c) -> q b c", c=512)[:, 0:nt, 0:TW]
                v = v1_pool.tile([128, 2 * TW], f16)
                nc.scalar.activation(v[:, 0:w], pav, IDENT,
                                     bias=sb1_sb[:, NG + g:NG + g + 1],
                                     scale=sb1_sb[:, g:g + 1])
                r0 = ROWS_PT * 2 * u + 1
                nc.vector.tensor_scalar(
                    h1v[p][g][:, r0:r0 + ROWS_PT * nt, 1:57], v[:, 0:w],
                    CLO, CHI, op0=mx, op1=mn)

            def emit_s2(i, g, u):
                p = i % 2
                nt = 2 if u < NU - 1 else NT - 2 * (NU - 1)
                w = TW * nt
                pb = pb_pool.tile([128, 1024], fp32)
                # tap-major: consecutive matmuls share lhsT
                for k, (dy, dx) in enumerate(TAPS):
                    lcol = 128 * (9 * g + k)
                    for j in range(nt):
                        t = 2 * u + j
                        r0 = ROWS_PT * t + 1
                        rhs = h1v[p][g][:, r0 + dy:r0 + dy + ROWS_PT,
                                        1 + dx:57 + dx]
                        nc.tensor.matmul(
                            pb[:, 512 * j:512 * j + TW],
                            wdw_sb[:, lcol:lcol + 128], rhs,
                            start=(k == 0), stop=(k == 8))
                pbv = pb[:].rearrange("q (b c) -> q b c", c=512)[:, 0:nt, 0:TW]
                v = v2_pool.tile([128, 2 * TW], f16)
                nc.scalar.activation(v[:, 0:w], pbv, IDENT,
                                     bias=b2m_sb[:, g:g + 1],
                                     scale=s2m_sb[:, g:g + 1])
                nc.vector.tensor_scalar(
                    h2t[p][g][:, 2 * TW * u:2 * TW * u + w], v[:, 0:w],
                    CLO, CHI, op0=mx, op1=mn)

            def emit_s2_vec(i, g, u, veng, acc_pool):
                # depthwise unit on DVE/GpSimd: 9 per-partition FMAs with
                # fp32 SBUF accumulation (exact: integer values), then the
                # usual ACT round + clamp.
                p = i % 2
                nt = 2 if u < NU - 1 else NT - 2 * (NU - 1)
                w = TW * nt
                rows = ROWS_PT * nt
                r0 = ROWS_PT * 2 * u + 1
                cur = acc_pool.tile([128, 2 * TW], fp32, tag="a", name="acc_a")
                nxt = acc_pool.tile([128, 2 * TW], fp32, tag="b", name="acc_b")
                for k, (dy, dx) in enumerate(TAPS):
                    win = h1v[p][g][:, r0 + dy:r0 + dy + rows, 1 + dx:57 + dx]
                    wap = wdv_sb[:, 9 * g + k:9 * g + k + 1]
                    if k == 0:
                        # first tap on ACT: w[c]*win with per-partition scale
                        nc.scalar.mul(cur[:, 0:w], win, wap)
                    else:
                        veng.scalar_tensor_tensor(nxt[:, 0:w], win, wap,
                                                  cur[:, 0:w],
                                                  op0=mult, op1=add)
                        cur, nxt = nxt, cur
                v = v2_pool.tile([128, 2 * TW], f16)
                nc.scalar.activation(v[:, 0:w], cur[:, 0:w], IDENT,
                                     bias=b2m_sb[:, g:g + 1],
                                     scale=s2m_sb[:, g:g + 1])
                veng.tensor_scalar(
                    h2t[p][g][:, 2 * TW * u:2 * TW * u + w], v[:, 0:w],
                    CLO, CHI, op0=mx, op1=mn)

            def emit_s3(i, t, xp_sb, o_sb):
                p = i % 2
                pc = pc_pool.tile([64, TW], fp32)
                for kc in range(NG):
                    nc.tensor.matmul(
                        pc[:], w3i_sb[:, 64 * kc:64 * (kc + 1)],
                        h2t[p][kc][:, TW * t:TW * (t + 1)],
                        start=(kc == 0), stop=(kc == NG - 1))
                v3 = v3_pool.tile([64, TW], f16, tag="v3", name="v3")
                nc.scalar.activation(v3[:], pc[:], IDENT,
                                     bias=b3m_sb[:, 0:1],
                                     scale=s3m_sb[:, 0:1])
                u3 = v3_pool.tile([64, TW], f16, tag="u3", name="u3")
                nc.vector.tensor_scalar(u3[:], v3[:], CLO, CHI,
                                        op0=mx, op1=mn)
                # out = (r3+1024)/255 + (x - 1024/255)
                nc.vector.scalar_tensor_tensor(
                    o_sb[:, TW * t:TW * (t + 1)], u3[:],
                    float(np.float32(1.0 / 255.0)),
                    xp_sb[:, TW * t:TW * (t + 1)],
                    op0=mult, op1=add)

            UNITS = [(g, u) for g in range(NG) for u in range(NU)]
            # image-0 xp is deferred behind the stage-2 weights (wdw must
            # land before the first s2 matmul ~13us in; xp isn't read
            # until stage 3 of image 0)
            bufs = {0: dma_in(0, split=True, defer_xp=True)}
            (wdw_sb, w3i_sb, s2m_sb, b2m_sb, s3m_sb, b3m_sb,
             wdv_sb) = late_consts()
            nc.sync.dma_start(bufs[0][1][:], xp[0, :, :])
            for (g, u) in UNITS:
                emit_s1(0, g, u, bufs[0][0])
            for i in range(BPC):
                last = i + 1 >= BPC
                if not last:
                    bufs[i + 1] = dma_in(i + 1)
                o_sb = o_pool.tile([64, PIX], fp32)
                dve_s2 = DVE_S2
                s3q = list(range(NT))

                def do_s3(i, t, o_sb):
                    emit_s3(i, t, bufs[i][1], o_sb)
                    if t == 3:
                        # stream output while the rest computes
                        nc.sync.dma_start(ys[i, :, 0:4 * TW],
                                          o_sb[:, 0:4 * TW])
                    elif t == 5:
                        nc.sync.dma_start(ys[i, :, 4 * TW:6 * TW],
                                          o_sb[:, 4 * TW:6 * TW])

                for j, (g, u) in enumerate(UNITS):
                    if (g, u) in GPS_S2:
                        emit_s2_vec(i, g, u, nc.gpsimd, accg_pool)
                    elif (g, u) in dve_s2:
                        emit_s2_vec(i, g, u, nc.vector, accd_pool)
                    else:
                        emit_s2(i, g, u)
                    if not last:
                        emit_s1(i + 1, g, u, bufs[i + 1][0])
                    elif j >= 9:
                        # last image: drain s3 early as h2 tiles complete
                        for t in (2 * (j - 9), 2 * (j - 9) + 1):
                            if t in s3q:
                                s3q.remove(t)
                                do_s3(i, t, o_sb)
                for t in s3q:
                    do_s3(i, t, o_sb)
                nc.sync.dma_start(ys[i, :, 6 * TW:PIX], o_sb[:, 6 * TW:PIX])
                del bufs[i]

    nc.compile()
    return nc


def _prep_weights(inputs):
    inv1 = (inputs['g1'] / np.sqrt(inputs['v1'] + EPS)).astype(np.float32)
    beta1 = (inputs['b1'] - inputs['m1'] * inv1).astype(np.float32)
    inv2 = (inputs['g2'] / np.sqrt(inputs['v2'] + EPS)).astype(np.float32)
    beta2 = (inputs['b2'] - inputs['m2'] * inv2).astype(np.float32)
    inv3 = (inputs['g3'] / np.sqrt(inputs['v3'] + EPS)).astype(np.float32)
    beta3 = (inputs['b3'] - inputs['m3'] * inv3).astype(np.float32)

    w1q = _quant_w(inputs['w1'])[:, :, 0, 0]       # [384, 64]
    w2q = _quant_w(inputs['w2'])[:, 0, :, :]       # [384, 3, 3]
    w3q = _quant_w(inputs['w3'])[:, :, 0, 0]       # [64, 384]

    # integer weights (w*255 is an exact odd integer <= 255, fp16-exact)
    w1i = np.round(w1q * 255.0).astype(np.float32).T        # [64, 384]
    w1s = np.concatenate([w1i, w1i], axis=0).astype(np.float16)  # [128, 384]

    wdw_int = np.round(w2q * 255.0).astype(np.float32)      # [384, 3, 3]
    wdw = np.zeros((128, NG * 9 * 128), np.float16)
    for g in range(NG):
        ch = slice(128 * g, 128 * (g + 1))
        k = 0
        for dy in range(3):
            for dx in range(3):
                col = 128 * (9 * g + k)
                wdw[:, col:col + 128][np.arange(128), np.arange(128)] = \
                    wdw_int[ch, dy, dx].astype(np.float16)
                k += 1

    w3int = np.round(w3q * 255.0).astype(np.float32)        # [64, 384]
    w3i = np.zeros((128, NG * 64), np.float16)
    for kc in range(NG):
        w3i[:, 64 * kc:64 * (kc + 1)] = \
            w3int[:, 128 * kc:128 * (kc + 1)].T.astype(np.float16)

    # stage-1: t1 = pa*(inv1/6) + beta1*42.5 ; +1024 fp16 magic
    s1 = (inv1 / np.float32(6.0)).astype(np.float32)
    b1 = (beta1 * np.float32(42.5) + OFF).astype(np.float32)
    s1m = s1.reshape(NG, 128).T.copy()
    b1m = b1.reshape(NG, 128).T.copy()

    # stage-2: inputs carry +1024; absorb 1024*sum(w2) into the bias
    sumw2 = wdw_int.sum(axis=(1, 2)).astype(np.float32)     # [384]
    s2 = (inv2 / np.float32(255.0)).astype(np.float32)
    b2 = (beta2 * np.float32(42.5) + OFF - s2 * OFF * sumw2).astype(np.float32)
    s2m = s2.reshape(NG, 128).T.copy()
    b2m = b2.reshape(NG, 128).T.copy()

    # stage-3: inputs carry +1024; absorb 1024*sum(w3) into the bias
    sumw3 = w3int.sum(axis=1).astype(np.float32)            # [64]
    s3 = (np.float32(6.0) * inv3 / np.float32(255.0)).astype(np.float32)
    b3 = (beta3 * np.float32(255.0) + OFF - s3 * OFF * sumw3).astype(np.float32)
    s3m = s3.reshape(64, 1)
    b3m = b3.reshape(64, 1)

    # per-partition tap weights for vector-engine depthwise units
    wdv = np.zeros((128, NG * 9), np.float32)
    for g in range(NG):
        k = 0
        for dy in range(3):
            for dx in range(3):
                wdv[:, 9 * g + k] = wdw_int[128 * g:128 * (g + 1), dy, dx]
                k += 1

    sb1 = np.concatenate([s1m, b1m], axis=1)
    return (w1s, np.ascontiguousarray(wdw), np.ascontiguousarray(w3i),
            np.ascontiguousarray(wdv), np.ascontiguousarray(sb1),
            np.ascontiguousarray(s2m), np.ascontiguousarray(b2m),
            np.ascontiguousarray(s3m), np.ascontiguousarray(b3m))


def _make_inmaps(inputs):
    (w1s, wdw, w3i, wdv, sb1, s2m, b2m, s3m, b3m) = _prep_weights(inputs)
    x = np.asarray(inputs['x'], np.float32).reshape(B, C, PIX)
    x_hi = x.astype(np.float16)
    x_lo = (x - x_hi.astype(np.float32)).astype(np.float16)
    xhl = np.concatenate([x_hi, x_lo], axis=1)              # [B, 128, PIX]
    xp = (x - np.float32(OFF / np.float32(255.0))).astype(np.float32)

    in_maps = []
    for c in range(NCORES):
        sl = slice(BPC * c, BPC * (c + 1))
        in_maps.append({'xhl': np.ascontiguousarray(xhl[sl]),
                        'xp': np.ascontiguousarray(xp[sl]),
                        'w1s': w1s, 'wdw': wdw, 'w3i': w3i, 'wdv': wdv,
                        'sb1': sb1, 's2m': s2m, 'b2m': b2m,
                        's3m': s3m, 'b3m': b3m})
    return in_maps


def kernel(**inputs):
    from concourse import bass_utils

    if 'nc' not in _cache:
        _cache['nc'] = _build_program()
    nc = _cache['nc']

    in_maps = _make_inmaps(inputs)
    res = bass_utils.run_bass_kernel_spmd(nc, in_maps, list(range(NCORES)))
    out = np.concatenate([res.results[c]['ys'] for c in range(NCORES)], axis=0)
    return out.reshape(B, C, H, W).astype(np.float32)
